# revision 7
# baseline (speedup 1.0000x reference)
"""Bi-directional correlation cost volume on 8 Trainium2 NeuronCores.

Strategy (data-parallel over batch, one batch element per core):
  - Host pre-scales L by 1/C and casts both inputs to bf16 (4x faster PE
    streaming than fp32, half the staging bytes; ~0.5% rel err, gate 2e-2).
  - Per core, compute the Gram band G[u, x] = sum_c L[c,h,u] * R[c,h,x]
    for |x - u| <= 63 with TensorE matmuls in five 64-row u-chunks
    (tight parallelogram windows: widths 127/190/190/190/127).
  - Chunk pairs share one 128-partition PSUM tile via tile_position
    column offsets (c0+c4, c1+c3, c2 packed across h-parity), so the
    PSUM->SBUF copies and the staging stores always run 128 partitions
    tall: DMA cost on TRN2 is per-partition free bytes, so half-height
    stores would waste half the bandwidth.
  - Stage bf16 to HBM as [128, h, w] per pair (contiguous h-group lines
    of 3.8-7.6 KB per partition); host extracts the 127 diagonals with
    one vectorized gather per batch.
"""

import numpy as np

B, C, H, WIMG, D = 8, 32, 160, 320, 64
HQ = H // 4   # h-rows per PE quadrant
HG = 20       # h-rows batched per store DMA
W0, W1, W2 = 127, 190, 190  # window widths: (c0,c4), (c1,c3), c2

_CACHE = {}


def _get_nc(reps=1):
    key = ("nc", reps)
    if key in _CACHE:
        return _CACHE[key]
    import concourse.bacc as bacc
    import concourse.tile as tile
    from concourse import mybir

    f32 = mybir.dt.float32
    bf16 = mybir.dt.bfloat16
    nc = bacc.Bacc("TRN2", target_bir_lowering=False, debug=False)
    r_in = nc.declare_dram_parameter("r_in", [C, H, WIMG], bf16, isOutput=False)
    l_in = nc.declare_dram_parameter("l_in", [C, H, WIMG], bf16, isOutput=False)
    # staged Gram band, bf16, partition-major:
    #   st0: P<64 -> c0 row u=P (x win [0,127)); P>=64 -> c4 row u=256+P-64
    #        (x win [193,320))
    #   st1: P<64 -> c1 row u=64+P ([1,191)); P>=64 -> c3 row u=192+P-64
    #        ([129,319))
    #   st2: P = 64*(h%2) + (u-128), index [P, h//2, x-65] ([65,255))
    st0 = nc.declare_dram_parameter("st0", [128, H, W0], bf16, isOutput=True)
    st1 = nc.declare_dram_parameter("st1", [128, H, W1], bf16, isOutput=True)
    st2 = nc.declare_dram_parameter("st2", [128, H // 2, W2], bf16, isOutput=True)

    with tile.TileContext(nc) as tc:
        with tc.tile_pool(name="inp", bufs=1) as inp_pool, \
             tc.tile_pool(name="ps04", bufs=3, space="PSUM") as ps04_pool, \
             tc.tile_pool(name="ps13", bufs=3, space="PSUM") as ps13_pool, \
             tc.tile_pool(name="ps2", bufs=2, space="PSUM") as ps2_pool, \
             tc.tile_pool(name="st", bufs=3) as st_pool:
            Lsb = inp_pool.tile([128, HQ * WIMG], bf16, tag="L")
            Rsb = inp_pool.tile([128, HQ * WIMG], bf16, tag="R")
            # partition (q, c) holds h-rows [40q, 40q+40) of channel c
            for q in range(4):
                nc.sync.dma_start(
                    Lsb[32 * q:32 * (q + 1), :],
                    l_in[:, HQ * q:HQ * (q + 1), :].rearrange(
                        "c hh x -> c (hh x)"),
                )
                nc.sync.dma_start(
                    Rsb[32 * q:32 * (q + 1), :],
                    r_in[:, HQ * q:HQ * (q + 1), :].rearrange(
                        "c hh x -> c (hh x)"),
                )
            for _ in range(reps):
                for q in range(4):
                    L = Lsb[32 * q:32 * (q + 1), :]
                    R = Rsb[32 * q:32 * (q + 1), :]
                    sb04 = sb13 = sb2 = None
                    ps04 = ps13 = ps2 = None
                    for hh in range(HQ):
                        base = hh * WIMG
                        slot = hh % HG
                        if slot == 0:
                            sb04 = st_pool.tile([128, HG * W0], bf16, tag="s0")
                            sb13 = st_pool.tile([128, HG * W1], bf16, tag="s1")
                            sb2 = st_pool.tile([128, (HG // 2) * W2], bf16,
                                               tag="s2")
                        par = hh % 2
                        s4 = hh % 4
                        # packed PSUM tiles: 4 h of (c0,c4), 2 h of (c1,c3),
                        # 4 h of c2 (parity partitions x 2 pair-slots) --
                        # fewer, larger PSUM->SBUF copies amortize the
                        # per-instruction PSUM read bubble.
                        if s4 == 0:
                            ps04 = ps04_pool.tile([128, 4 * W0], f32, tag="p0")
                            ps2 = ps2_pool.tile([128, 2 * W2], f32, tag="p2")
                        if par == 0:
                            ps13 = ps13_pool.tile([128, 2 * W1], f32, tag="p1")
                        nc.tensor.matmul(
                            ps04[0:64, s4 * W0:(s4 + 1) * W0],
                            L[:, base:base + 64],
                            R[:, base:base + W0],
                            start=True, stop=True, tile_position=(32 * q, 0))
                        nc.tensor.matmul(
                            ps04[64:128, s4 * W0:(s4 + 1) * W0],
                            L[:, base + 256:base + 320],
                            R[:, base + 193:base + 193 + W0],
                            start=True, stop=True, tile_position=(32 * q, 64))
                        nc.tensor.matmul(
                            ps13[0:64, par * W1:(par + 1) * W1],
                            L[:, base + 64:base + 128],
                            R[:, base + 1:base + 1 + W1],
                            start=True, stop=True, tile_position=(32 * q, 0))
                        nc.tensor.matmul(
                            ps13[64:128, par * W1:(par + 1) * W1],
                            L[:, base + 192:base + 256],
                            R[:, base + 129:base + 129 + W1],
                            start=True, stop=True, tile_position=(32 * q, 64))
                        nc.tensor.matmul(
                            ps2[64 * par:64 * par + 64,
                                (s4 // 2) * W2:(s4 // 2 + 1) * W2],
                            L[:, base + 128:base + 192],
                            R[:, base + 65:base + 65 + W2],
                            start=True, stop=True,
                            tile_position=(32 * q, 64 * par))
                        # PSUM -> SBUF cast copies, balanced DVE/Act
                        if par == 1:
                            dst13 = sb13[:, (slot - 1) * W1:(slot + 1) * W1]
                            if (hh // 2) in (2, 9, 16):
                                nc.vector.tensor_scalar_add(dst13, ps13, 0.0)
                            else:
                                nc.scalar.copy(dst13, ps13)
                        if s4 == 3:
                            nc.vector.tensor_scalar_add(
                                sb04[:, (slot - 3) * W0:(slot + 1) * W0],
                                ps04, 0.0)
                            dst2 = sb2[:, ((slot - 3) // 2) * W2:
                                       (((slot - 3) // 2) + 2) * W2]
                            if hh % 8 == 3:
                                nc.vector.tensor_scalar_add(dst2, ps2, 0.0)
                            else:
                                nc.scalar.copy(dst2, ps2)
                        if slot == HG - 1:
                            h0 = HQ * q + hh - (HG - 1)
                            nc.sync.dma_start(
                                st0[:, h0:h0 + HG, :],
                                sb04.rearrange("p (g w) -> p g w", g=HG))
                            nc.scalar.dma_start(
                                st1[:, h0:h0 + HG, :],
                                sb13.rearrange("p (g w) -> p g w", g=HG))
                            nc.sync.dma_start(
                                st2[:, h0 // 2:h0 // 2 + HG // 2, :],
                                sb2.rearrange("p (g w) -> p g w", g=HG // 2))
    nc.compile()
    _CACHE[key] = nc
    return nc


def _gather_idx():
    """Flat indices into concat(st0, st1, st2) for out[plane, h, x]."""
    if "idx" in _CACHE:
        return _CACHE["idx"]
    N0 = 128 * H * W0
    N1 = 128 * H * W1
    P = np.arange(2 * D)[:, None, None]
    dts = np.where(P < D, P, -(P - D))  # signed disparity per output plane
    h = np.arange(H)[None, :, None]
    x = np.arange(WIMG)[None, None, :]
    u = np.clip(x - dts, 0, WIMG - 1) + 0 * h  # broadcast to [2D, H, W]
    h = h + 0 * u
    xx = x + 0 * u
    idx = np.empty(u.shape, dtype=np.int64)
    m = u < 64
    idx[m] = ((u * H + h) * W0 + xx)[m]
    m = (u >= 64) & (u < 128)
    idx[m] = (N0 + (((u - 64) * H + h) * W1 + (xx - 1)))[m]
    m = (u >= 128) & (u < 192)
    idx[m] = (N0 + N1 +
              (((64 * (h % 2) + u - 128) * (H // 2) + h // 2) * W2 +
               (xx - 65)))[m]
    m = (u >= 192) & (u < 256)
    idx[m] = (N0 + (((64 + u - 192) * H + h) * W1 + (xx - 129)))[m]
    m = u >= 256
    idx[m] = (((64 + u - 256) * H + h) * W0 + (xx - 193))[m]
    _CACHE["idx"] = np.ascontiguousarray(idx)
    return _CACHE["idx"]


def _assemble(st0, st1, st2):
    """staged bf16 arrays -> out_b [2D, H, WIMG] float32"""
    idx = _gather_idx()
    flat = np.concatenate([
        np.asarray(st0).astype(np.float32).ravel(),
        np.asarray(st1).astype(np.float32).ravel(),
        np.asarray(st2).astype(np.float32).ravel(),
    ])
    o = np.take(flat, idx)
    for d in range(1, D):
        o[d, :, :d] = 0
        o[D + d, :, WIMG - d:] = 0
    return o


def _to_bf16(a):
    import ml_dtypes
    return np.ascontiguousarray(a.astype(ml_dtypes.bfloat16))


def run_cores(right_np, left_np, timing_reps=0):
    """Run the SPMD bass kernel; returns list of (st0, st1, st2) per batch."""
    from concourse.bass_utils import run_bass_kernel_spmd

    nc = _get_nc()
    rb = _to_bf16(right_np)
    lb = _to_bf16(left_np * (1.0 / C))
    in_maps = [{"r_in": rb[b], "l_in": lb[b]} for b in range(B)]
    res = run_bass_kernel_spmd(nc, in_maps, list(range(B)))
    return [(res.results[b]["st0"], res.results[b]["st1"],
             res.results[b]["st2"]) for b in range(B)]


def kernel(right_feature, left_feature, max_disp):
    assert int(max_disp) == D
    right_np = np.asarray(right_feature, dtype=np.float32)
    left_np = np.asarray(left_feature, dtype=np.float32)
    stags = run_cores(right_np, left_np)
    out = np.stack([_assemble(*s) for s in stags])
    return out
